# revision 33
# baseline (speedup 1.0000x reference)
"""Trainium2 Bass kernel for nn_Entangle: y, s = entangle(x, masks, ...).

Sharding: tensor-parallel over the signal axis S (16 signals / 8 cores = 2
signals per core).  All the heavy work — the [B,S,C,N,N] masked
superposition tensor `s` and the collapse/smear path for `y` — runs on
device.  The tiny [S,S] correlation statistic reduces exactly to
corr[i,j] = sum_{b,c} d_bi d_bj / (B*C*N) with d = x.sum(-1)  (the mean
over the irfft time axis keeps only the DC bin), so the per-signal mix
weights are folded host-side into three per-row scalars.

Device math per core (units u = (s_local, channel), rows r = (u, b)):
  sig = fft(x): two real matmuls with DFT cos/-sin matrices, in both row
    [r, k] and transposed [k, r] layouts (no PE transposes anywhere)
  P = a(x)a - b(x)b, Q = a(x)b + b(x)a: K=2 outer products on PE; both
    operands of each outer product live at partitions {0,1} of a flat
    region (matmul operands must share an aligned start partition)
  s = P*mask_re - Q*mask_im: DVE multiplies (PSUM reads) + GPSIMD subtract
  R = M@sig, Rt = M^T@sig in transposed layout: mask tiles stationary,
    sigT columns moving, fused [A|B] / [-B|A] 16-col stationaries put
    Re/Im in adjacent psum columns
  smear = truncated-irfft basis matmuls of sig (.) R   (PE)
  y = lam1*smear_r + lam2*smear_c + lam3*x  (fused scalar_tensor_tensor)
"""

import numpy as np

B, S, C, N = 8, 16, 4, 256
NCORES = 8
SIG_PER_CORE = S // NCORES          # 2 signals per core
UNITS = SIG_PER_CORE * C            # 8 (s_local, c) units per core
ROWS = UNITS * B                    # 64 rows per core
BGROUP = 4                          # batch rows per P/Q psum tile


def _host_constants():
    """DFT + truncated-irfft basis matrices, float32."""
    n = np.arange(N, dtype=np.float64)
    k = n[:, None] * n[None, :] * (2.0 * np.pi / N)
    cos = np.cos(k)
    nsin = -np.sin(k)
    # irfft of half-spectrum H[0..128]:
    # y[t] = (1/N)[ReH0 + 2*sum_{1..127}(ReHk cos(2pi k t/N) - ImHk sin) + ReH128 cos(pi t)]
    kk = np.arange(N)[:, None].astype(np.float64)
    tt = np.arange(N)[None, :].astype(np.float64)
    wr = np.cos(2 * np.pi * kk * tt / N) / N * np.where((kk % (N // 2)) == 0, 1.0, 2.0)
    wi = -np.sin(2 * np.pi * kk * tt / N) / N * 2.0
    wr[129:] = 0.0
    wi[129:] = 0.0
    wi[0] = 0.0
    wi[128] = 0.0
    return (cos.astype(np.float32), nsin.astype(np.float32),
            wr.astype(np.float32), wi.astype(np.float32))


def build_program():
    """Build the (SPMD, per-core identical) Bass program. Returns nc."""
    import concourse.bass as bass
    import concourse.tile as tile
    from concourse import mybir

    f32 = mybir.dt.float32
    op = mybir.AluOpType

    nc = bass.Bass("TRN2", target_bir_lowering=False, debug=False)

    # ---- DRAM I/O ----
    t = {}
    t["xr_d"] = nc.dram_tensor("xr", [ROWS, N], f32, kind="ExternalInput")
    t["xrt_d"] = nc.dram_tensor("xrt", [N, ROWS], f32, kind="ExternalInput")
    t["mr_d"] = nc.dram_tensor("mr", [UNITS, N, N], f32, kind="ExternalInput")
    t["mi_d"] = nc.dram_tensor("mi", [UNITS, N, N], f32, kind="ExternalInput")
    t["mrt_d"] = nc.dram_tensor("mrt", [UNITS, N, N], f32, kind="ExternalInput")
    t["mit_d"] = nc.dram_tensor("mit", [UNITS, N, N], f32, kind="ExternalInput")
    t["cos_d"] = nc.dram_tensor("cosm", [N, N], f32, kind="ExternalInput")
    t["nsin_d"] = nc.dram_tensor("nsinm", [N, N], f32, kind="ExternalInput")
    t["wr_d"] = nc.dram_tensor("wrm", [N, N], f32, kind="ExternalInput")
    t["wi_d"] = nc.dram_tensor("wim", [N, N], f32, kind="ExternalInput")
    t["lam_d"] = nc.dram_tensor("lam", [ROWS, 4], f32, kind="ExternalInput")
    t["s_d"] = nc.dram_tensor("s_out", [UNITS, B, N, N], f32, kind="ExternalOutput")
    t["y_d"] = nc.dram_tensor("y_out", [ROWS, N], f32, kind="ExternalOutput")

    dummy_sem = nc.alloc_semaphore("mmwait_dummy")
    with tile.TileContext(nc) as tc:
        _emit(tc, t, f32, op)

    # Post-pass over the scheduled BIR:
    #  1. drop PE-on-PE waits from matmuls (the engine runs its queue in
    #     order and matmuls complete pc-monotonically — redundant);
    #  2. walrus's Matmult codegen (S3_LW struct) cannot carry
    #     compute-semaphore waits at all, so hoist any remaining
    #     non-DMA waits onto a standalone EventSemaphore op right
    #     before the matmul (same engine, so ordering is preserved).
    nwait = 0
    for bb in nc.m.functions[0].blocks:
        new_insts = []
        for inst in bb.instructions:
            ty = type(inst).__name__
            si = inst.sync_info
            if si and si.on_wait:
                ws = list(si.on_wait)
                if ty == "InstMatmult":
                    ws = [w for w in ws
                          if not (w.ant_name or "").startswith("PE_")]
                    keep = [w for w in ws if (w.ant_name or "").startswith("DMA")][:1]
                    hoist = [w for w in ws if w not in keep]
                elif ty == "InstEventSemaphore":
                    keep, hoist = ws, []
                else:
                    keep, hoist = ws[:1], ws[1:]
                while hoist:
                    chunk, hoist = hoist[:2], hoist[2:]
                    nwait += 1
                    upd = mybir.SyncUpdate(
                        sync_type="semaphore", id=dummy_sem.num,
                        ant_name=dummy_sem.name, update_mode="sem-add-imm",
                        update_value=1)
                    new_insts.append(mybir.InstEventSemaphore(
                        name=f"I-mmwait-{nwait}",
                        engine=inst.engine,
                        ins=[], outs=[],
                        sync_info=mybir.SyncInfo(on_wait=chunk, on_update=[upd]),
                    ))
                if len(keep) != len(si.on_wait):
                    inst.sync_info = mybir.SyncInfo(
                        on_wait=keep, on_update=si.on_update)
            new_insts.append(inst)
        bb.instructions = new_insts
    return nc


def _emit(tc, t, f32, op):
    nc = tc.nc
    import contextlib
    ctx = contextlib.ExitStack()
    with ctx:
        consts = ctx.enter_context(tc.tile_pool(name="consts", bufs=1))
        sigp = ctx.enter_context(tc.tile_pool(name="sig", bufs=1))
        maskp = ctx.enter_context(tc.tile_pool(name="masks", bufs=1))
        masktp = ctx.enter_context(tc.tile_pool(name="maskt", bufs=2))
        ttp = ctx.enter_context(tc.tile_pool(name="tt", bufs=3))
        sp = ctx.enter_context(tc.tile_pool(name="sout", bufs=4))

        # ---------- constants ----------
        def load_c(name, dram, rows, cols):
            tl = consts.tile([rows, cols], f32, tag=name, name=name)
            nc.sync.dma_start(tl[:], dram)
            return tl

        COS = [load_c(f"cos{i}", t["cos_d"][i * 128:(i + 1) * 128, :], 128, N)
               for i in range(2)]
        NSIN = [load_c(f"nsin{i}", t["nsin_d"][i * 128:(i + 1) * 128, :], 128, N)
                for i in range(2)]
        WR = [load_c(f"wr{i}", t["wr_d"][i * 128:(i + 1) * 128, :], 128, N)
              for i in range(2)]
        WI = [load_c(f"wi{i}", t["wi_d"][i * 128:(i + 1) * 128, :], 128, N)
              for i in range(2)]
        XRT = [load_c(f"xrt{i}", t["xrt_d"][i * 128:(i + 1) * 128, :], 128, ROWS)
               for i in range(2)]
        LAM = load_c("lam", t["lam_d"][:, :], ROWS, 4)
        XROW = load_c("xrow", t["xr_d"][:, :], ROWS, N)

        # ---------- Stage A: FFT (row + transposed layouts) ----------
        stA = tc.tile_pool(name="psA", bufs=1, space="PSUM")
        psA = stA.__enter__()

        A_ps = psA.tile([ROWS, N], f32, tag="Aps", name="Aps")
        B_ps = psA.tile([ROWS, N], f32, tag="Bps", name="Bps")
        nc.tensor.matmul(A_ps[:], XRT[0][:], COS[0][:], start=True, stop=False)
        nc.tensor.matmul(A_ps[:], XRT[1][:], COS[1][:], start=False, stop=True)
        nc.tensor.matmul(B_ps[:], XRT[0][:], NSIN[0][:], start=True, stop=False)
        nc.tensor.matmul(B_ps[:], XRT[1][:], NSIN[1][:], start=False, stop=True)

        # transposed spectra: AT[k, r] = sum_n cos[n, k] x[r, n], lhsT = COS as stored
        ATp, BTp = [], []
        for kt in range(2):
            at = psA.tile([128, ROWS], f32, tag=f"ATps{kt}", name=f"ATps{kt}")
            bt = psA.tile([128, ROWS], f32, tag=f"BTps{kt}", name=f"BTps{kt}")
            ksl = slice(kt * 128, (kt + 1) * 128)
            nc.tensor.matmul(at[:], COS[0][:, ksl], XRT[0][:], start=True, stop=False)
            nc.tensor.matmul(at[:], COS[1][:, ksl], XRT[1][:], start=False, stop=True)
            nc.tensor.matmul(bt[:], NSIN[0][:, ksl], XRT[0][:], start=True, stop=False)
            nc.tensor.matmul(bt[:], NSIN[1][:, ksl], XRT[1][:], start=False, stop=True)
            ATp.append(at)
            BTp.append(bt)

        # SIGT[kt]: [128, 256] cols = [AT(64) | BT(64) | -BT(64) | AT(64)]
        SIGT = [sigp.tile([128, 256], f32, tag=f"SIGT{i}", name=f"SIGT{i}")
                for i in range(2)]
        for kt in range(2):
            nc.scalar.copy(SIGT[kt][:, 0:64], ATp[kt][:])
            nc.scalar.copy(SIGT[kt][:, 64:128], BTp[kt][:])
            nc.scalar.mul(SIGT[kt][:, 128:192], BTp[kt][:], -1.0)
            nc.scalar.copy(SIGT[kt][:, 192:256], ATp[kt][:])

        # Row-major staging for the outer-product pair regions.  Partition 0
        # of the flat half-tensor FLH holds, per row r, [a | a | b]; partition
        # 1 holds [b | -b | a], giving pairs {a,b}, {a,-b}, {b,a} at free
        # offsets (3*loc + pair) * N.
        SROW = sigp.tile([ROWS, 3 * N], f32, tag="SROW", name="SROW")
        SROW2 = sigp.tile([ROWS, 3 * N], f32, tag="SROW2", name="SROW2")
        for dst_t, i, src_ps, neg in (
                (SROW, 0, A_ps, False), (SROW, 1, A_ps, False),
                (SROW, 2, B_ps, False),
                (SROW2, 0, B_ps, False), (SROW2, 1, B_ps, True),
                (SROW2, 2, A_ps, False)):
            dst = dst_t[:, i * N:(i + 1) * N]
            if neg:
                nc.scalar.mul(dst, src_ps[:], -1.0)
            else:
                nc.scalar.copy(dst, src_ps[:])

        # gate matmul: waits on the LAST Act copy (SROW2 block 2), which
        # absorbs every earlier Act tick (SIGT/SROW copies) into PE's
        # observed clock so later matmuls don't re-wait on Activation.
        gp = psA.tile([1, 256], f32, tag="gateA", name="gateA")
        nc.tensor.matmul(gp[:], SROW2[:, 512:513], SROW2[:, 512:768],
                         start=True, stop=True)
        stA.__exit__(None, None, None)

        # ---------- Stage C: matvecs in transposed layout ----------
        # psum PS[o][pb] [128, 128]: unit u owns cols u*16..u*16+16,
        # cols u*16..+8 = Re part, +8..+16 = Im part.
        #   R  (o=0): lhsT = mrt/mit tiles, Re = mr@a - mi@b, Im = mr@b + mi@a
        #   Rt (o=1): lhsT = mr/mi tiles (contraction over p)
        stC = tc.tile_pool(name="psC", bufs=1, space="PSUM")
        psC = stC.__enter__()
        MASKS = {}
        PS = [[psC.tile([128, 128], f32, tag=f"ps{o}{pb}", name=f"ps{o}{pb}")
               for pb in range(2)] for o in range(2)]

        def ab16(u):
            # 16-col stationary view [A-8 | B-8] of SIGT
            return lambda kt: SIGT[kt][:].rearrange(
                "p (g c) -> p g c", g=4)[:, 0:2, u * B:u * B + B]

        def nba16(u):
            # [-B-8 | A2-8]
            return lambda kt: SIGT[kt][:].rearrange(
                "p (g c) -> p g c", g=4)[:, 2:4, u * B:u * B + B]

        for u in range(UNITS):
            # each mask loads with ONE dma into [128, 512]: cols kt*256+q
            mrt = masktp.tile([128, 512], f32, tag="mrt", name="mrt")
            mit = masktp.tile([128, 512], f32, tag="mit", name="mit")
            mrk = maskp.tile([128, 512], f32, tag=f"mr_{u}", name=f"mr_{u}")
            mik = maskp.tile([128, 512], f32, tag=f"mi_{u}", name=f"mi_{u}")
            for tile_, dram in ((mrt, t["mrt_d"]), (mit, t["mit_d"]),
                                (mrk, t["mr_d"]), (mik, t["mi_d"])):
                nc.sync.dma_start(
                    tile_[:].rearrange("p (kt q) -> p kt q", kt=2),
                    dram[u].rearrange("(kt p) q -> p kt q", kt=2))
                # ifmap-side gate: absorbs this tile's DMA tick into PE's
                # observed clock (MM-side wait; LDW stays clean for later mms)
                gp = psC.tile([1, 512], f32, tag="gateC", name="gateC", bufs=2)
                nc.tensor.matmul(gp[:], SIGT[0][:, 0:1], tile_[:],
                                 start=True, stop=True)
            MASKS[(u, 0, "r")] = mrk
            MASKS[(u, 0, "i")] = mik
            ucol = slice(u * 16, u * 16 + 16)
            for o, (m_re, m_im) in enumerate(((mrt, mit), (mrk, mik))):
                for pb in range(2):
                    out = PS[o][pb][:, ucol]
                    for kt in range(2):
                        pbsl = slice(kt * 256 + pb * 128, kt * 256 + pb * 128 + 128)
                        nc.tensor.matmul(out, m_re[:, pbsl], ab16(u)(kt),
                                         start=(kt == 0), stop=False)
                    for kt in range(2):
                        pbsl = slice(kt * 256 + pb * 128, kt * 256 + pb * 128 + 128)
                        nc.tensor.matmul(out, m_im[:, pbsl], nba16(u)(kt),
                                         start=False, stop=(kt == 1))

        # ---------- Stage D: spectra products + irfft + y ----------
        # ReRST[kt] = AT (.) Re - BT (.) Im ; ImRST[kt] = AT (.) Im + BT (.) Re
        tmp1 = sigp.tile([128, 64], f32, tag="tmp1", name="tmp1")
        tmp2 = sigp.tile([128, 64], f32, tag="tmp2", name="tmp2")
        RST = {}
        for o, nm in ((0, "RS"), (1, "CS")):
            for kt in range(2):
                atv = SIGT[kt][:, 0:64].rearrange("p (u c) -> p u c", c=8)
                btv = SIGT[kt][:, 64:128].rearrange("p (u c) -> p u c", c=8)
                pv = PS[o][kt][:].rearrange("p (u h c) -> p u h c", h=2, c=8)
                re_v, im_v = pv[:, :, 0, :], pv[:, :, 1, :]
                ret = sigp.tile([128, 64], f32, tag=f"re{nm}{kt}", name=f"re{nm}{kt}")
                imt = sigp.tile([128, 64], f32, tag=f"im{nm}{kt}", name=f"im{nm}{kt}")
                t1v = tmp1[:].rearrange("p (u c) -> p u c", c=8)
                t2v = tmp2[:].rearrange("p (u c) -> p u c", c=8)
                nc.vector.tensor_mul(t1v, atv, re_v)
                nc.vector.tensor_mul(t2v, btv, im_v)
                nc.vector.tensor_sub(ret[:], tmp1[:], tmp2[:])
                nc.vector.tensor_mul(t1v, atv, im_v)
                nc.vector.tensor_mul(t2v, btv, re_v)
                nc.vector.tensor_add(imt[:], tmp1[:], tmp2[:])
                RST[(nm, kt)] = (ret, imt)
        stC.__exit__(None, None, None)

        stD = tc.tile_pool(name="psD", bufs=1, space="PSUM")
        psD = stD.__enter__()
        sm_r = psD.tile([ROWS, N], f32, tag="smr", name="smr")
        sm_c = psD.tile([ROWS, N], f32, tag="smc", name="smc")
        for ps, nm in ((sm_r, "RS"), (sm_c, "CS")):
            nc.tensor.matmul(ps[:], RST[(nm, 0)][0][:], WR[0][:], start=True, stop=False)
            nc.tensor.matmul(ps[:], RST[(nm, 1)][0][:], WR[1][:], start=False, stop=False)
            nc.tensor.matmul(ps[:], RST[(nm, 0)][1][:], WI[0][:], start=False, stop=False)
            nc.tensor.matmul(ps[:], RST[(nm, 1)][1][:], WI[1][:], start=False, stop=True)

        G = sigp.tile([ROWS, N], f32, tag="G", name="G")
        G2 = sigp.tile([ROWS, N], f32, tag="G2", name="G2")
        YR = sigp.tile([ROWS, N], f32, tag="YR", name="YR")
        nc.vector.tensor_scalar(G[:], sm_r[:], LAM[:, 0:1], None, op.mult)
        nc.vector.scalar_tensor_tensor(G2[:], sm_c[:], LAM[:, 1:2], G[:],
                                       op.mult, op.add)
        nc.vector.scalar_tensor_tensor(YR[:], XROW[:], LAM[:, 2:3], G2[:],
                                       op.mult, op.add)
        nc.sync.dma_start(t["y_d"][:, :], YR[:])

        # First flat-half fill + gate, emitted inside the stage-D psum scope
        # so the gate lands on a fresh bank (no pool-boundary WAR waits).
        HROWS = ROWS // 2

        def fill_flh(h, pool, tag):
            rows = slice(h * HROWS, (h + 1) * HROWS)
            nc.sync.dma_start(FLH[0:1, :], SROW[rows, :])
            nc.sync.dma_start(FLH[1:2, :], SROW2[rows, :])
            gp = pool.tile([1, 256], f32, tag=tag, name="gate_flh", bufs=2)
            nc.tensor.matmul(gp[:], FLH[:, 0:1], FLH[:, 0:256],
                             start=True, stop=True)

        FLH = sigp.tile([2, HROWS * 3 * N], f32, tag="FLH", name="FLH")
        SROW_, SROW2_ = SROW, SROW2
        fill_flh(0, psD, "gateD")
        stD.__exit__(None, None, None)

        # ---------- Stage E: superposition s ----------
        # Flat pair regions (refilled per unit, one DMA per partition):
        # both operands of every outer product start at partition 0.
        #   P = a@a - b@b:  lhsT = {a,b} (region 0),  rhs = {a,-b} (region 1)
        #   Q = a@b + b@a:  lhsT = {a,b},             rhs = {b,a}  (region 2)
        stE = tc.tile_pool(name="psPQ", bufs=2, space="PSUM")
        psPQ = stE.__enter__()
        ngroup = B // BGROUP
        FD = BGROUP * N

        for u in range(UNITS):
            if u == 4:
                fill_flh(1, psPQ, "gateE")
            r0 = u * B
            for pt in range(2):
                for g in range(ngroup):
                    P4 = psPQ.tile([128, FD], f32, tag="P", name="P")
                    Q4 = psPQ.tile([128, FD], f32, tag="Q", name="Q", bufs=1)
                    for j in range(BGROUP):
                        b = g * BGROUP + j
                        loc = (u % 4) * B + b          # row within this FLH half
                        base = loc * 3 * N
                        lsl = slice(base + pt * 128, base + pt * 128 + 128)
                        r1 = slice(base + N, base + 2 * N)
                        r2 = slice(base + 2 * N, base + 3 * N)
                        osl = slice(j * N, (j + 1) * N)
                        nc.tensor.matmul(P4[:, osl], FLH[:, lsl], FLH[:, r1],
                                         start=True, stop=True, tile_position=(0, 0))
                        nc.tensor.matmul(Q4[:, osl], FLH[:, lsl], FLH[:, r2],
                                         start=True, stop=True, tile_position=(0, 0))
                    # mask p-tile pt lives at free cols pt*256.. of the
                    # [128, 512] combined tile
                    mr_pt = MASKS[(u, 0, "r")][:, pt * N:(pt + 1) * N]
                    mi_pt = MASKS[(u, 0, "i")][:, pt * N:(pt + 1) * N]
                    mrb = mr_pt.unsqueeze(1).broadcast_to((128, BGROUP, N))
                    mib = mi_pt.unsqueeze(1).broadcast_to((128, BGROUP, N))
                    t1 = ttp.tile([128, FD], f32, tag="t1", name="t1")
                    t2 = ttp.tile([128, FD], f32, tag="t2", name="t2")
                    nc.vector.tensor_tensor(
                        t1[:].rearrange("p (b q) -> p b q", b=BGROUP),
                        P4[:].rearrange("p (b q) -> p b q", b=BGROUP), mrb, op.mult)
                    nc.vector.tensor_tensor(
                        t2[:].rearrange("p (b q) -> p b q", b=BGROUP),
                        Q4[:].rearrange("p (b q) -> p b q", b=BGROUP), mib, op.mult)
                    s4 = sp.tile([128, FD], f32, tag="s", name="s")
                    nc.gpsimd.tensor_sub(s4[:], t1[:], t2[:])
                    dst = t["s_d"][u, g * BGROUP:(g + 1) * BGROUP,
                                   pt * 128:(pt + 1) * 128, :].rearrange(
                                       "b p q -> p b q")
                    nc.sync.dma_start(dst, s4[:].rearrange("p (b q) -> p b q",
                                                           b=BGROUP))
        stE.__exit__(None, None, None)


_NC_CACHE = None


def _get_nc():
    global _NC_CACHE
    if _NC_CACHE is None:
        _NC_CACHE = build_program()
    return _NC_CACHE


def make_in_maps(x, mask_real, mask_imag, polarization, gauss_mean, gauss_std):
    """Host-side sharding + tiny exact statistics."""
    x = np.asarray(x, np.float32)
    mr_f = np.asarray(mask_real, np.float32)
    mi_f = np.asarray(mask_imag, np.float32)
    pol = np.asarray(polarization, np.float64)
    gm = np.asarray(gauss_mean, np.float64)
    gs = np.asarray(gauss_std, np.float64)

    cos, nsin, wr, wi = _host_constants()

    # exact correlation statistic (DC-bin reduction of the irfft mean)
    d = x.astype(np.float64).sum(-1)                       # [B,S,C]
    corr = np.einsum('bic,bjc->ij', d, d) / (B * C * N)    # [S,S]
    mix = np.exp(-0.5 * ((corr - gm[:, None]) / gs[:, None]) ** 2)
    mixbar = mix.mean(1)                                   # [S]
    lam1 = (mixbar * np.cos(pol)).astype(np.float32)
    lam2 = (mixbar * np.sin(pol)).astype(np.float32)
    lam3 = (1.0 - mixbar).astype(np.float32)

    in_maps = []
    for core in range(NCORES):
        s0 = core * SIG_PER_CORE
        xs = x[:, s0:s0 + SIG_PER_CORE]                    # [B,2,C,N]
        xrow = np.ascontiguousarray(
            xs.transpose(1, 2, 0, 3).reshape(ROWS, N))     # rows r=(sl,c,b)
        xrt = np.ascontiguousarray(xrow.T)                 # [N, ROWS]
        mru = np.ascontiguousarray(
            mr_f[s0:s0 + SIG_PER_CORE].reshape(UNITS, N, N))
        miu = np.ascontiguousarray(
            mi_f[s0:s0 + SIG_PER_CORE].reshape(UNITS, N, N))
        mrtu = np.ascontiguousarray(mru.transpose(0, 2, 1))
        mitu = np.ascontiguousarray(miu.transpose(0, 2, 1))
        sl_idx = np.repeat(np.arange(SIG_PER_CORE), C * B) + s0   # [64] signal ids
        lam = np.stack([lam1[sl_idx], lam2[sl_idx], lam3[sl_idx],
                        np.zeros(ROWS, np.float32)], axis=1).astype(np.float32)
        in_maps.append({
            "xr": xrow, "xrt": xrt, "mr": mru, "mi": miu, "mrt": mrtu,
            "mit": mitu, "cosm": cos, "nsinm": nsin, "wrm": wr, "wim": wi,
            "lam": lam,
        })
    return in_maps


def assemble(results):
    """Gather per-core outputs into full (y, s)."""
    y = np.empty((B, S, C, N), np.float32)
    s = np.empty((B, S, C, N, N), np.float32)
    for core, res in enumerate(results):
        s0 = core * SIG_PER_CORE
        yr = res["y_out"].reshape(SIG_PER_CORE, C, B, N)
        y[:, s0:s0 + SIG_PER_CORE] = yr.transpose(2, 0, 1, 3)
        so = res["s_out"].reshape(SIG_PER_CORE, C, B, N, N)
        s[:, s0:s0 + SIG_PER_CORE] = so.transpose(2, 0, 1, 3, 4)
    return y, s


def kernel(x, mask_real, mask_imag, polarization, gauss_mean, gauss_std):
    from concourse.bass_utils import run_bass_kernel_spmd
    nc = _get_nc()
    in_maps = make_in_maps(x, mask_real, mask_imag, polarization,
                           gauss_mean, gauss_std)
    res = run_bass_kernel_spmd(nc, in_maps, core_ids=list(range(NCORES)))
    return assemble(res.results)


# revision 34
# speedup vs baseline: 1.0117x; 1.0117x over previous
"""Trainium2 Bass kernel for nn_Entangle: y, s = entangle(x, masks, ...).

Sharding: tensor-parallel over the signal axis S (16 signals / 8 cores = 2
signals per core).  All the heavy work — the [B,S,C,N,N] masked
superposition tensor `s` and the collapse/smear path for `y` — runs on
device.  The tiny [S,S] correlation statistic reduces exactly to
corr[i,j] = sum_{b,c} d_bi d_bj / (B*C*N) with d = x.sum(-1)  (the mean
over the irfft time axis keeps only the DC bin), so the per-signal mix
weights are folded host-side into three per-row scalars.

Device math per core (units u = (s_local, channel), rows r = (u, b)):
  sig = fft(x): two real matmuls with DFT cos/-sin matrices, in both row
    [r, k] and transposed [k, r] layouts (no PE transposes anywhere)
  P = a(x)a - b(x)b, Q = a(x)b + b(x)a: K=2 outer products on PE; both
    operands of each outer product live at partitions {0,1} of a flat
    region (matmul operands must share an aligned start partition)
  s = P*mask_re - Q*mask_im: DVE multiplies (PSUM reads) + GPSIMD subtract
  R = M@sig, Rt = M^T@sig in transposed layout: mask tiles stationary,
    sigT columns moving, fused [A|B] / [-B|A] 16-col stationaries put
    Re/Im in adjacent psum columns
  smear = truncated-irfft basis matmuls of sig (.) R   (PE)
  y = lam1*smear_r + lam2*smear_c + lam3*x  (fused scalar_tensor_tensor)
"""

import numpy as np

B, S, C, N = 8, 16, 4, 256
NCORES = 8
SIG_PER_CORE = S // NCORES          # 2 signals per core
UNITS = SIG_PER_CORE * C            # 8 (s_local, c) units per core
ROWS = UNITS * B                    # 64 rows per core
BGROUP = 4                          # batch rows per P/Q psum tile


def _host_constants():
    """DFT + truncated-irfft basis matrices, float32."""
    n = np.arange(N, dtype=np.float64)
    k = n[:, None] * n[None, :] * (2.0 * np.pi / N)
    cos = np.cos(k)
    nsin = -np.sin(k)
    # irfft of half-spectrum H[0..128]:
    # y[t] = (1/N)[ReH0 + 2*sum_{1..127}(ReHk cos(2pi k t/N) - ImHk sin) + ReH128 cos(pi t)]
    kk = np.arange(N)[:, None].astype(np.float64)
    tt = np.arange(N)[None, :].astype(np.float64)
    wr = np.cos(2 * np.pi * kk * tt / N) / N * np.where((kk % (N // 2)) == 0, 1.0, 2.0)
    wi = -np.sin(2 * np.pi * kk * tt / N) / N * 2.0
    wr[129:] = 0.0
    wi[129:] = 0.0
    wi[0] = 0.0
    wi[128] = 0.0
    return (cos.astype(np.float32), nsin.astype(np.float32),
            wr.astype(np.float32), wi.astype(np.float32))


def build_program():
    """Build the (SPMD, per-core identical) Bass program. Returns nc."""
    import concourse.bass as bass
    import concourse.tile as tile
    from concourse import mybir

    f32 = mybir.dt.float32
    op = mybir.AluOpType

    nc = bass.Bass("TRN2", target_bir_lowering=False, debug=False)

    # ---- DRAM I/O ----
    t = {}
    t["xr_d"] = nc.dram_tensor("xr", [ROWS, N], f32, kind="ExternalInput")
    t["xrt_d"] = nc.dram_tensor("xrt", [N, ROWS], f32, kind="ExternalInput")
    t["mr_d"] = nc.dram_tensor("mr", [UNITS, N, N], f32, kind="ExternalInput")
    t["mi_d"] = nc.dram_tensor("mi", [UNITS, N, N], f32, kind="ExternalInput")
    t["mrt_d"] = nc.dram_tensor("mrt", [UNITS, N, N], f32, kind="ExternalInput")
    t["mit_d"] = nc.dram_tensor("mit", [UNITS, N, N], f32, kind="ExternalInput")
    t["cos_d"] = nc.dram_tensor("cosm", [N, N], f32, kind="ExternalInput")
    t["nsin_d"] = nc.dram_tensor("nsinm", [N, N], f32, kind="ExternalInput")
    t["wr_d"] = nc.dram_tensor("wrm", [N, N], f32, kind="ExternalInput")
    t["wi_d"] = nc.dram_tensor("wim", [N, N], f32, kind="ExternalInput")
    t["lam_d"] = nc.dram_tensor("lam", [ROWS, 4], f32, kind="ExternalInput")
    t["s_d"] = nc.dram_tensor("s_out", [UNITS, B, N, N], f32, kind="ExternalOutput")
    t["y_d"] = nc.dram_tensor("y_out", [ROWS, N], f32, kind="ExternalOutput")

    dummy_sem = nc.alloc_semaphore("mmwait_dummy")
    with tile.TileContext(nc) as tc:
        _emit(tc, t, f32, op)

    # Post-pass over the scheduled BIR:
    #  1. drop PE-on-PE waits from matmuls (the engine runs its queue in
    #     order and matmuls complete pc-monotonically — redundant);
    #  2. walrus's Matmult codegen (S3_LW struct) cannot carry
    #     compute-semaphore waits at all, so hoist any remaining
    #     non-DMA waits onto a standalone EventSemaphore op right
    #     before the matmul (same engine, so ordering is preserved).
    nwait = 0
    for bb in nc.m.functions[0].blocks:
        new_insts = []
        for inst in bb.instructions:
            ty = type(inst).__name__
            si = inst.sync_info
            if si and si.on_wait:
                ws = list(si.on_wait)
                if ty == "InstMatmult":
                    ws = [w for w in ws
                          if not (w.ant_name or "").startswith("PE_")]
                    keep = [w for w in ws if (w.ant_name or "").startswith("DMA")][:1]
                    hoist = [w for w in ws if w not in keep]
                elif ty == "InstEventSemaphore":
                    keep, hoist = ws, []
                else:
                    keep, hoist = ws[:1], ws[1:]
                while hoist:
                    chunk, hoist = hoist[:2], hoist[2:]
                    nwait += 1
                    upd = mybir.SyncUpdate(
                        sync_type="semaphore", id=dummy_sem.num,
                        ant_name=dummy_sem.name, update_mode="sem-add-imm",
                        update_value=1)
                    new_insts.append(mybir.InstEventSemaphore(
                        name=f"I-mmwait-{nwait}",
                        engine=inst.engine,
                        ins=[], outs=[],
                        sync_info=mybir.SyncInfo(on_wait=chunk, on_update=[upd]),
                    ))
                if len(keep) != len(si.on_wait):
                    inst.sync_info = mybir.SyncInfo(
                        on_wait=keep, on_update=si.on_update)
            new_insts.append(inst)
        bb.instructions = new_insts
    return nc


def _emit(tc, t, f32, op):
    nc = tc.nc
    import contextlib
    ctx = contextlib.ExitStack()
    with ctx:
        consts = ctx.enter_context(tc.tile_pool(name="consts", bufs=1))
        sigp = ctx.enter_context(tc.tile_pool(name="sig", bufs=1))
        maskp = ctx.enter_context(tc.tile_pool(name="masks", bufs=1))
        masktp = ctx.enter_context(tc.tile_pool(name="maskt", bufs=2))
        ttp = ctx.enter_context(tc.tile_pool(name="tt", bufs=3))
        sp = ctx.enter_context(tc.tile_pool(name="sout", bufs=4))

        # ---------- constants ----------
        def load_c(name, dram, rows, cols):
            tl = consts.tile([rows, cols], f32, tag=name, name=name)
            nc.sync.dma_start(tl[:], dram)
            return tl

        COS = [load_c(f"cos{i}", t["cos_d"][i * 128:(i + 1) * 128, :], 128, N)
               for i in range(2)]
        NSIN = [load_c(f"nsin{i}", t["nsin_d"][i * 128:(i + 1) * 128, :], 128, N)
                for i in range(2)]
        WR = [load_c(f"wr{i}", t["wr_d"][i * 128:(i + 1) * 128, :], 128, N)
              for i in range(2)]
        WI = [load_c(f"wi{i}", t["wi_d"][i * 128:(i + 1) * 128, :], 128, N)
              for i in range(2)]
        XRT = [load_c(f"xrt{i}", t["xrt_d"][i * 128:(i + 1) * 128, :], 128, ROWS)
               for i in range(2)]
        LAM = load_c("lam", t["lam_d"][:, :], ROWS, 4)
        XROW = load_c("xrow", t["xr_d"][:, :], ROWS, N)

        # ---------- Stage A: FFT (row + transposed layouts) ----------
        stA = tc.tile_pool(name="psA", bufs=1, space="PSUM")
        psA = stA.__enter__()

        A_ps = psA.tile([ROWS, N], f32, tag="Aps", name="Aps")
        B_ps = psA.tile([ROWS, N], f32, tag="Bps", name="Bps")
        nc.tensor.matmul(A_ps[:], XRT[0][:], COS[0][:], start=True, stop=False)
        nc.tensor.matmul(A_ps[:], XRT[1][:], COS[1][:], start=False, stop=True)
        nc.tensor.matmul(B_ps[:], XRT[0][:], NSIN[0][:], start=True, stop=False)
        nc.tensor.matmul(B_ps[:], XRT[1][:], NSIN[1][:], start=False, stop=True)

        # transposed spectra: AT[k, r] = sum_n cos[n, k] x[r, n], lhsT = COS as stored
        ATp, BTp = [], []
        for kt in range(2):
            at = psA.tile([128, ROWS], f32, tag=f"ATps{kt}", name=f"ATps{kt}")
            bt = psA.tile([128, ROWS], f32, tag=f"BTps{kt}", name=f"BTps{kt}")
            ksl = slice(kt * 128, (kt + 1) * 128)
            nc.tensor.matmul(at[:], COS[0][:, ksl], XRT[0][:], start=True, stop=False)
            nc.tensor.matmul(at[:], COS[1][:, ksl], XRT[1][:], start=False, stop=True)
            nc.tensor.matmul(bt[:], NSIN[0][:, ksl], XRT[0][:], start=True, stop=False)
            nc.tensor.matmul(bt[:], NSIN[1][:, ksl], XRT[1][:], start=False, stop=True)
            ATp.append(at)
            BTp.append(bt)

        # SIGT[kt]: [128, 256] cols = [AT(64) | BT(64) | -BT(64) | AT(64)]
        SIGT = [sigp.tile([128, 256], f32, tag=f"SIGT{i}", name=f"SIGT{i}")
                for i in range(2)]
        for kt in range(2):
            nc.scalar.copy(SIGT[kt][:, 0:64], ATp[kt][:])
            nc.scalar.copy(SIGT[kt][:, 64:128], BTp[kt][:])
            nc.scalar.mul(SIGT[kt][:, 128:192], BTp[kt][:], -1.0)
            nc.scalar.copy(SIGT[kt][:, 192:256], ATp[kt][:])

        # Row-major staging for the outer-product pair regions.  Partition 0
        # of the flat half-tensor FLH holds, per row r, [a | a | b]; partition
        # 1 holds [b | -b | a], giving pairs {a,b}, {a,-b}, {b,a} at free
        # offsets (3*loc + pair) * N.
        SROW = sigp.tile([ROWS, 3 * N], f32, tag="SROW", name="SROW")
        SROW2 = sigp.tile([ROWS, 3 * N], f32, tag="SROW2", name="SROW2")
        for dst_t, i, src_ps, neg in (
                (SROW, 0, A_ps, False), (SROW, 1, A_ps, False),
                (SROW, 2, B_ps, False),
                (SROW2, 0, B_ps, False), (SROW2, 1, B_ps, True),
                (SROW2, 2, A_ps, False)):
            dst = dst_t[:, i * N:(i + 1) * N]
            if neg:
                nc.scalar.mul(dst, src_ps[:], -1.0)
            else:
                nc.scalar.copy(dst, src_ps[:])

        # gate matmul: waits on the LAST Act copy (SROW2 block 2), which
        # absorbs every earlier Act tick (SIGT/SROW copies) into PE's
        # observed clock so later matmuls don't re-wait on Activation.
        gp = psA.tile([1, 256], f32, tag="gateA", name="gateA")
        nc.tensor.matmul(gp[:], SROW2[:, 512:513], SROW2[:, 512:768],
                         start=True, stop=True)
        stA.__exit__(None, None, None)

        # ---------- Stage C: matvecs in transposed layout ----------
        # psum PS[o][pb] [128, 128]: unit u owns cols u*16..u*16+16,
        # cols u*16..+8 = Re part, +8..+16 = Im part.
        #   R  (o=0): lhsT = mrt/mit tiles, Re = mr@a - mi@b, Im = mr@b + mi@a
        #   Rt (o=1): lhsT = mr/mi tiles (contraction over p)
        stC = tc.tile_pool(name="psC", bufs=1, space="PSUM")
        psC = stC.__enter__()
        MASKS = {}
        PS = [[psC.tile([128, 128], f32, tag=f"ps{o}{pb}", name=f"ps{o}{pb}")
               for pb in range(2)] for o in range(2)]

        def ab16(u):
            # 16-col stationary view [A-8 | B-8] of SIGT
            return lambda kt: SIGT[kt][:].rearrange(
                "p (g c) -> p g c", g=4)[:, 0:2, u * B:u * B + B]

        def nba16(u):
            # [-B-8 | A2-8]
            return lambda kt: SIGT[kt][:].rearrange(
                "p (g c) -> p g c", g=4)[:, 2:4, u * B:u * B + B]

        for u in range(UNITS):
            # each mask loads with ONE dma into [128, 512]: cols kt*256+q
            mrt = masktp.tile([128, 512], f32, tag="mrt", name="mrt")
            mit = masktp.tile([128, 512], f32, tag="mit", name="mit")
            mrk = maskp.tile([128, 512], f32, tag=f"mr_{u}", name=f"mr_{u}")
            mik = maskp.tile([128, 512], f32, tag=f"mi_{u}", name=f"mi_{u}")
            for tile_, dram in ((mrt, t["mrt_d"]), (mit, t["mit_d"]),
                                (mrk, t["mr_d"]), (mik, t["mi_d"])):
                nc.sync.dma_start(
                    tile_[:].rearrange("p (kt q) -> p kt q", kt=2),
                    dram[u].rearrange("(kt p) q -> p kt q", kt=2))
                # ifmap-side gate: absorbs this tile's DMA tick into PE's
                # observed clock (MM-side wait; LDW stays clean for later mms)
                gp = psC.tile([1, 512], f32, tag="gateC", name="gateC", bufs=2)
                nc.tensor.matmul(gp[:], SIGT[0][:, 0:1], tile_[:],
                                 start=True, stop=True)
            MASKS[(u, 0, "r")] = mrk
            MASKS[(u, 0, "i")] = mik
            ucol = slice(u * 16, u * 16 + 16)
            for o, (m_re, m_im) in enumerate(((mrt, mit), (mrk, mik))):
                for pb in range(2):
                    out = PS[o][pb][:, ucol]
                    for kt in range(2):
                        pbsl = slice(kt * 256 + pb * 128, kt * 256 + pb * 128 + 128)
                        nc.tensor.matmul(out, m_re[:, pbsl], ab16(u)(kt),
                                         start=(kt == 0), stop=False)
                    for kt in range(2):
                        pbsl = slice(kt * 256 + pb * 128, kt * 256 + pb * 128 + 128)
                        nc.tensor.matmul(out, m_im[:, pbsl], nba16(u)(kt),
                                         start=False, stop=(kt == 1))

        # ---------- Stage D: spectra products + irfft + y ----------
        # ReRST[kt] = AT (.) Re - BT (.) Im ; ImRST[kt] = AT (.) Im + BT (.) Re
        tmp1 = sigp.tile([128, 64], f32, tag="tmp1", name="tmp1")
        tmp2 = sigp.tile([128, 64], f32, tag="tmp2", name="tmp2")
        RST = {}
        for o, nm in ((0, "RS"), (1, "CS")):
            for kt in range(2):
                atv = SIGT[kt][:, 0:64].rearrange("p (u c) -> p u c", c=8)
                btv = SIGT[kt][:, 64:128].rearrange("p (u c) -> p u c", c=8)
                pv = PS[o][kt][:].rearrange("p (u h c) -> p u h c", h=2, c=8)
                re_v, im_v = pv[:, :, 0, :], pv[:, :, 1, :]
                ret = sigp.tile([128, 64], f32, tag=f"re{nm}{kt}", name=f"re{nm}{kt}")
                imt = sigp.tile([128, 64], f32, tag=f"im{nm}{kt}", name=f"im{nm}{kt}")
                t1v = tmp1[:].rearrange("p (u c) -> p u c", c=8)
                t2v = tmp2[:].rearrange("p (u c) -> p u c", c=8)
                nc.vector.tensor_mul(t1v, atv, re_v)
                nc.vector.tensor_mul(t2v, btv, im_v)
                nc.vector.tensor_sub(ret[:], tmp1[:], tmp2[:])
                nc.vector.tensor_mul(t1v, atv, im_v)
                nc.vector.tensor_mul(t2v, btv, re_v)
                nc.vector.tensor_add(imt[:], tmp1[:], tmp2[:])
                RST[(nm, kt)] = (ret, imt)
        stC.__exit__(None, None, None)

        stD = tc.tile_pool(name="psD", bufs=1, space="PSUM")
        psD = stD.__enter__()
        sm_r = psD.tile([ROWS, N], f32, tag="smr", name="smr")
        sm_c = psD.tile([ROWS, N], f32, tag="smc", name="smc")
        for ps, nm in ((sm_r, "RS"), (sm_c, "CS")):
            nc.tensor.matmul(ps[:], RST[(nm, 0)][0][:], WR[0][:], start=True, stop=False)
            nc.tensor.matmul(ps[:], RST[(nm, 1)][0][:], WR[1][:], start=False, stop=False)
            nc.tensor.matmul(ps[:], RST[(nm, 0)][1][:], WI[0][:], start=False, stop=False)
            nc.tensor.matmul(ps[:], RST[(nm, 1)][1][:], WI[1][:], start=False, stop=True)

        G = sigp.tile([ROWS, N], f32, tag="G", name="G")
        G2 = sigp.tile([ROWS, N], f32, tag="G2", name="G2")
        YR = sigp.tile([ROWS, N], f32, tag="YR", name="YR")
        nc.vector.tensor_scalar(G[:], sm_r[:], LAM[:, 0:1], None, op.mult)
        nc.vector.scalar_tensor_tensor(G2[:], sm_c[:], LAM[:, 1:2], G[:],
                                       op.mult, op.add)
        nc.vector.scalar_tensor_tensor(YR[:], XROW[:], LAM[:, 2:3], G2[:],
                                       op.mult, op.add)
        nc.sync.dma_start(t["y_d"][:, :], YR[:])

        # Block-diagonal operand tensors for the outer products (stage E).
        # P_b[p,q] = sum_k lhsT[k,p] rhs[k,q-block b] with k = 16 rows:
        #   rows 0:8  = a_b'[.]   (diag block b' of rhs / column p of lhsT)
        #   rows 8:16 = -b_b'[.] (P)   or   a/b swapped (Q)
        # One K=16, N=512 matmul then yields P (or Q) for TWO batch rows.
        # rhs RD[u%2]: [16, 2048] block-diagonal, refilled per unit by 16
        # small DMAs (zeros persist across refills since slots alternate).
        RD = [sigp.tile([16, B * N], f32, tag=f"RD{i}", name=f"RD{i}")
              for i in range(2)]
        LTP = [sigp.tile([16, N], f32, tag=f"LTP{i}", name=f"LTP{i}")
               for i in range(2)]
        LTQ = [sigp.tile([16, N], f32, tag=f"LTQ{i}", name=f"LTQ{i}")
               for i in range(2)]
        for i in range(2):
            nc.vector.memset(RD[i][:], 0.0)
        stD.__exit__(None, None, None)

        # ---------- Stage E: superposition s ----------
        # Flat pair regions (refilled per unit, one DMA per partition):
        # both operands of every outer product start at partition 0.
        #   P = a@a - b@b:  lhsT = {a,b} (region 0),  rhs = {a,-b} (region 1)
        #   Q = a@b + b@a:  lhsT = {a,b},             rhs = {b,a}  (region 2)
        stE = tc.tile_pool(name="psPQ", bufs=2, space="PSUM")
        psPQ = stE.__enter__()
        ngroup = B // BGROUP
        FD = BGROUP * N

        for u in range(UNITS):
            r0 = u * B
            rd, ltp, ltq = RD[u % 2], LTP[u % 2], LTQ[u % 2]
            # refill block-diagonal rhs + stationaries for this unit
            for bb_ in range(B):
                r = r0 + bb_
                nc.sync.dma_start(rd[bb_:bb_ + 1, bb_ * N:(bb_ + 1) * N],
                                  SROW[r:r + 1, 0:N])
                nc.sync.dma_start(rd[8 + bb_:9 + bb_, bb_ * N:(bb_ + 1) * N],
                                  SROW[r:r + 1, 2 * N:3 * N])
            nc.sync.dma_start(ltp[0:8, :], SROW[r0:r0 + B, 0:N])
            nc.sync.dma_start(ltp[8:16, :], SROW2[r0:r0 + B, N:2 * N])
            nc.sync.dma_start(ltq[0:8, :], SROW[r0:r0 + B, 2 * N:3 * N])
            nc.sync.dma_start(ltq[8:16, :], SROW[r0:r0 + B, 0:N])
            for pt in range(2):
                psl = slice(pt * 128, pt * 128 + 128)
                for g in range(ngroup):
                    P4 = psPQ.tile([128, FD], f32, tag="P", name="P")
                    Q4 = psPQ.tile([128, FD], f32, tag="Q", name="Q", bufs=1)
                    for h in range(2):
                        rsl = slice(g * FD + h * 512, g * FD + (h + 1) * 512)
                        osl = slice(h * 512, (h + 1) * 512)
                        nc.tensor.matmul(P4[:, osl], ltp[:, psl], rd[:, rsl],
                                         start=True, stop=True,
                                         tile_position=(0, 0))
                        nc.tensor.matmul(Q4[:, osl], ltq[:, psl], rd[:, rsl],
                                         start=True, stop=True,
                                         tile_position=(0, 0))
                    mr_pt = MASKS[(u, 0, "r")][:, pt * N:(pt + 1) * N]
                    mi_pt = MASKS[(u, 0, "i")][:, pt * N:(pt + 1) * N]
                    mrb = mr_pt.unsqueeze(1).broadcast_to((128, BGROUP, N))
                    mib = mi_pt.unsqueeze(1).broadcast_to((128, BGROUP, N))
                    t1 = ttp.tile([128, FD], f32, tag="t1", name="t1")
                    t2 = ttp.tile([128, FD], f32, tag="t2", name="t2")
                    nc.vector.tensor_tensor(
                        t1[:].rearrange("p (b q) -> p b q", b=BGROUP),
                        P4[:].rearrange("p (b q) -> p b q", b=BGROUP), mrb, op.mult)
                    nc.vector.tensor_tensor(
                        t2[:].rearrange("p (b q) -> p b q", b=BGROUP),
                        Q4[:].rearrange("p (b q) -> p b q", b=BGROUP), mib, op.mult)
                    s4 = sp.tile([128, FD], f32, tag="s", name="s")
                    nc.gpsimd.tensor_sub(s4[:], t1[:], t2[:])
                    dst = t["s_d"][u, g * BGROUP:(g + 1) * BGROUP,
                                   pt * 128:(pt + 1) * 128, :].rearrange(
                                       "b p q -> p b q")
                    nc.sync.dma_start(dst, s4[:].rearrange("p (b q) -> p b q",
                                                           b=BGROUP))
        stE.__exit__(None, None, None)


_NC_CACHE = None


def _get_nc():
    global _NC_CACHE
    if _NC_CACHE is None:
        _NC_CACHE = build_program()
    return _NC_CACHE


def make_in_maps(x, mask_real, mask_imag, polarization, gauss_mean, gauss_std):
    """Host-side sharding + tiny exact statistics."""
    x = np.asarray(x, np.float32)
    mr_f = np.asarray(mask_real, np.float32)
    mi_f = np.asarray(mask_imag, np.float32)
    pol = np.asarray(polarization, np.float64)
    gm = np.asarray(gauss_mean, np.float64)
    gs = np.asarray(gauss_std, np.float64)

    cos, nsin, wr, wi = _host_constants()

    # exact correlation statistic (DC-bin reduction of the irfft mean)
    d = x.astype(np.float64).sum(-1)                       # [B,S,C]
    corr = np.einsum('bic,bjc->ij', d, d) / (B * C * N)    # [S,S]
    mix = np.exp(-0.5 * ((corr - gm[:, None]) / gs[:, None]) ** 2)
    mixbar = mix.mean(1)                                   # [S]
    lam1 = (mixbar * np.cos(pol)).astype(np.float32)
    lam2 = (mixbar * np.sin(pol)).astype(np.float32)
    lam3 = (1.0 - mixbar).astype(np.float32)

    in_maps = []
    for core in range(NCORES):
        s0 = core * SIG_PER_CORE
        xs = x[:, s0:s0 + SIG_PER_CORE]                    # [B,2,C,N]
        xrow = np.ascontiguousarray(
            xs.transpose(1, 2, 0, 3).reshape(ROWS, N))     # rows r=(sl,c,b)
        xrt = np.ascontiguousarray(xrow.T)                 # [N, ROWS]
        mru = np.ascontiguousarray(
            mr_f[s0:s0 + SIG_PER_CORE].reshape(UNITS, N, N))
        miu = np.ascontiguousarray(
            mi_f[s0:s0 + SIG_PER_CORE].reshape(UNITS, N, N))
        mrtu = np.ascontiguousarray(mru.transpose(0, 2, 1))
        mitu = np.ascontiguousarray(miu.transpose(0, 2, 1))
        sl_idx = np.repeat(np.arange(SIG_PER_CORE), C * B) + s0   # [64] signal ids
        lam = np.stack([lam1[sl_idx], lam2[sl_idx], lam3[sl_idx],
                        np.zeros(ROWS, np.float32)], axis=1).astype(np.float32)
        in_maps.append({
            "xr": xrow, "xrt": xrt, "mr": mru, "mi": miu, "mrt": mrtu,
            "mit": mitu, "cosm": cos, "nsinm": nsin, "wrm": wr, "wim": wi,
            "lam": lam,
        })
    return in_maps


def assemble(results):
    """Gather per-core outputs into full (y, s)."""
    y = np.empty((B, S, C, N), np.float32)
    s = np.empty((B, S, C, N, N), np.float32)
    for core, res in enumerate(results):
        s0 = core * SIG_PER_CORE
        yr = res["y_out"].reshape(SIG_PER_CORE, C, B, N)
        y[:, s0:s0 + SIG_PER_CORE] = yr.transpose(2, 0, 1, 3)
        so = res["s_out"].reshape(SIG_PER_CORE, C, B, N, N)
        s[:, s0:s0 + SIG_PER_CORE] = so.transpose(2, 0, 1, 3, 4)
    return y, s


def kernel(x, mask_real, mask_imag, polarization, gauss_mean, gauss_std):
    from concourse.bass_utils import run_bass_kernel_spmd
    nc = _get_nc()
    in_maps = make_in_maps(x, mask_real, mask_imag, polarization,
                           gauss_mean, gauss_std)
    res = run_bass_kernel_spmd(nc, in_maps, core_ids=list(range(NCORES)))
    return assemble(res.results)


# revision 36
# speedup vs baseline: 1.1540x; 1.1406x over previous
"""Trainium2 Bass kernel for nn_Entangle: y, s = entangle(x, masks, ...).

Sharding: tensor-parallel over the signal axis S (16 signals / 8 cores = 2
signals per core).  All the heavy work — the [B,S,C,N,N] masked
superposition tensor `s` and the collapse/smear path for `y` — runs on
device.  The tiny [S,S] correlation statistic reduces exactly to
corr[i,j] = sum_{b,c} d_bi d_bj / (B*C*N) with d = x.sum(-1)  (the mean
over the irfft time axis keeps only the DC bin), so the per-signal mix
weights are folded host-side into three per-row scalars.

Device math per core (units u = (s_local, channel), rows r = (u, b)):
  sig = fft(x): two real matmuls with DFT cos/-sin matrices, in both row
    [r, k] and transposed [k, r] layouts (no PE transposes anywhere)
  P = a(x)a - b(x)b, Q = a(x)b + b(x)a: K=2 outer products on PE; both
    operands of each outer product live at partitions {0,1} of a flat
    region (matmul operands must share an aligned start partition)
  s = P*mask_re - Q*mask_im: DVE multiplies (PSUM reads) + GPSIMD subtract
  R = M@sig, Rt = M^T@sig in transposed layout: mask tiles stationary,
    sigT columns moving, fused [A|B] / [-B|A] 16-col stationaries put
    Re/Im in adjacent psum columns
  smear = truncated-irfft basis matmuls of sig (.) R   (PE)
  y = lam1*smear_r + lam2*smear_c + lam3*x  (fused scalar_tensor_tensor)
"""

import numpy as np

B, S, C, N = 8, 16, 4, 256
NCORES = 8
SIG_PER_CORE = S // NCORES          # 2 signals per core
UNITS = SIG_PER_CORE * C            # 8 (s_local, c) units per core
ROWS = UNITS * B                    # 64 rows per core
BGROUP = 4                          # batch rows per P/Q psum tile


def _host_constants():
    """DFT + truncated-irfft basis matrices, float32."""
    n = np.arange(N, dtype=np.float64)
    k = n[:, None] * n[None, :] * (2.0 * np.pi / N)
    cos = np.cos(k)
    nsin = -np.sin(k)
    # irfft of half-spectrum H[0..128]:
    # y[t] = (1/N)[ReH0 + 2*sum_{1..127}(ReHk cos(2pi k t/N) - ImHk sin) + ReH128 cos(pi t)]
    kk = np.arange(N)[:, None].astype(np.float64)
    tt = np.arange(N)[None, :].astype(np.float64)
    wr = np.cos(2 * np.pi * kk * tt / N) / N * np.where((kk % (N // 2)) == 0, 1.0, 2.0)
    wi = -np.sin(2 * np.pi * kk * tt / N) / N * 2.0
    wr[129:] = 0.0
    wi[129:] = 0.0
    wi[0] = 0.0
    wi[128] = 0.0
    return (cos.astype(np.float32), nsin.astype(np.float32),
            wr.astype(np.float32), wi.astype(np.float32))


def build_program():
    """Build the (SPMD, per-core identical) Bass program. Returns nc."""
    import concourse.bass as bass
    import concourse.tile as tile
    from concourse import mybir

    f32 = mybir.dt.float32
    op = mybir.AluOpType

    nc = bass.Bass("TRN2", target_bir_lowering=False, debug=False)

    # ---- DRAM I/O ----
    t = {}
    t["xr_d"] = nc.dram_tensor("xr", [ROWS, N], f32, kind="ExternalInput")
    t["xrt_d"] = nc.dram_tensor("xrt", [N, ROWS], f32, kind="ExternalInput")
    t["mr_d"] = nc.dram_tensor("mr", [UNITS, N, N], f32, kind="ExternalInput")
    t["mi_d"] = nc.dram_tensor("mi", [UNITS, N, N], f32, kind="ExternalInput")
    t["mrt_d"] = nc.dram_tensor("mrt", [UNITS, N, N], f32, kind="ExternalInput")
    t["mit_d"] = nc.dram_tensor("mit", [UNITS, N, N], f32, kind="ExternalInput")
    t["cos_d"] = nc.dram_tensor("cosm", [N, N], f32, kind="ExternalInput")
    t["nsin_d"] = nc.dram_tensor("nsinm", [N, N], f32, kind="ExternalInput")
    t["wr_d"] = nc.dram_tensor("wrm", [N, N], f32, kind="ExternalInput")
    t["wi_d"] = nc.dram_tensor("wim", [N, N], f32, kind="ExternalInput")
    t["lam_d"] = nc.dram_tensor("lam", [ROWS, 4], f32, kind="ExternalInput")
    t["s_d"] = nc.dram_tensor("s_out", [UNITS, B, N, N], f32, kind="ExternalOutput")
    t["y_d"] = nc.dram_tensor("y_out", [ROWS, N], f32, kind="ExternalOutput")

    dummy_sem = nc.alloc_semaphore("mmwait_dummy")
    with tile.TileContext(nc) as tc:
        _emit(tc, t, f32, op)

    # Post-pass over the scheduled BIR:
    #  1. drop PE-on-PE waits from matmuls (the engine runs its queue in
    #     order and matmuls complete pc-monotonically — redundant);
    #  2. walrus's Matmult codegen (S3_LW struct) cannot carry
    #     compute-semaphore waits at all, so hoist any remaining
    #     non-DMA waits onto a standalone EventSemaphore op right
    #     before the matmul (same engine, so ordering is preserved).
    nwait = 0
    for bb in nc.m.functions[0].blocks:
        new_insts = []
        for inst in bb.instructions:
            ty = type(inst).__name__
            si = inst.sync_info
            if si and si.on_wait:
                ws = list(si.on_wait)
                if ty == "InstMatmult":
                    ws = [w for w in ws
                          if not (w.ant_name or "").startswith("PE_")]
                    keep = [w for w in ws if (w.ant_name or "").startswith("DMA")][:1]
                    hoist = [w for w in ws if w not in keep]
                elif ty == "InstEventSemaphore":
                    keep, hoist = ws, []
                else:
                    keep, hoist = ws[:1], ws[1:]
                while hoist:
                    chunk, hoist = hoist[:2], hoist[2:]
                    nwait += 1
                    upd = mybir.SyncUpdate(
                        sync_type="semaphore", id=dummy_sem.num,
                        ant_name=dummy_sem.name, update_mode="sem-add-imm",
                        update_value=1)
                    new_insts.append(mybir.InstEventSemaphore(
                        name=f"I-mmwait-{nwait}",
                        engine=inst.engine,
                        ins=[], outs=[],
                        sync_info=mybir.SyncInfo(on_wait=chunk, on_update=[upd]),
                    ))
                if len(keep) != len(si.on_wait):
                    inst.sync_info = mybir.SyncInfo(
                        on_wait=keep, on_update=si.on_update)
            new_insts.append(inst)
        bb.instructions = new_insts
    return nc


def _emit(tc, t, f32, op):
    nc = tc.nc
    import contextlib
    ctx = contextlib.ExitStack()
    with ctx:
        consts = ctx.enter_context(tc.tile_pool(name="consts", bufs=1))
        sigp = ctx.enter_context(tc.tile_pool(name="sig", bufs=1))
        maskp = ctx.enter_context(tc.tile_pool(name="masks", bufs=1))
        masktp = ctx.enter_context(tc.tile_pool(name="maskt", bufs=2))
        ttp = ctx.enter_context(tc.tile_pool(name="tt", bufs=3))
        sp = ctx.enter_context(tc.tile_pool(name="sout", bufs=4))

        # ---------- constants ----------
        def load_c(name, dram, rows, cols):
            tl = consts.tile([rows, cols], f32, tag=name, name=name)
            nc.sync.dma_start(tl[:], dram)
            return tl

        COS = [load_c(f"cos{i}", t["cos_d"][i * 128:(i + 1) * 128, :], 128, N)
               for i in range(2)]
        NSIN = [load_c(f"nsin{i}", t["nsin_d"][i * 128:(i + 1) * 128, :], 128, N)
                for i in range(2)]
        WR = [load_c(f"wr{i}", t["wr_d"][i * 128:(i + 1) * 128, :], 128, N)
              for i in range(2)]
        WI = [load_c(f"wi{i}", t["wi_d"][i * 128:(i + 1) * 128, :], 128, N)
              for i in range(2)]
        XRT = [load_c(f"xrt{i}", t["xrt_d"][i * 128:(i + 1) * 128, :], 128, ROWS)
               for i in range(2)]
        LAM = load_c("lam", t["lam_d"][:, :], ROWS, 4)
        XROW = load_c("xrow", t["xr_d"][:, :], ROWS, N)

        # ---------- Stage A: FFT (row + transposed layouts) ----------
        stA = tc.tile_pool(name="psA", bufs=1, space="PSUM")
        psA = stA.__enter__()

        A_ps = psA.tile([ROWS, N], f32, tag="Aps", name="Aps")
        B_ps = psA.tile([ROWS, N], f32, tag="Bps", name="Bps")
        nc.tensor.matmul(A_ps[:], XRT[0][:], COS[0][:], start=True, stop=False)
        nc.tensor.matmul(A_ps[:], XRT[1][:], COS[1][:], start=False, stop=True)
        nc.tensor.matmul(B_ps[:], XRT[0][:], NSIN[0][:], start=True, stop=False)
        nc.tensor.matmul(B_ps[:], XRT[1][:], NSIN[1][:], start=False, stop=True)

        # transposed spectra: AT[k, r] = sum_n cos[n, k] x[r, n], lhsT = COS as stored
        ATp, BTp = [], []
        for kt in range(2):
            at = psA.tile([128, ROWS], f32, tag=f"ATps{kt}", name=f"ATps{kt}")
            bt = psA.tile([128, ROWS], f32, tag=f"BTps{kt}", name=f"BTps{kt}")
            ksl = slice(kt * 128, (kt + 1) * 128)
            nc.tensor.matmul(at[:], COS[0][:, ksl], XRT[0][:], start=True, stop=False)
            nc.tensor.matmul(at[:], COS[1][:, ksl], XRT[1][:], start=False, stop=True)
            nc.tensor.matmul(bt[:], NSIN[0][:, ksl], XRT[0][:], start=True, stop=False)
            nc.tensor.matmul(bt[:], NSIN[1][:, ksl], XRT[1][:], start=False, stop=True)
            ATp.append(at)
            BTp.append(bt)

        # SIGT[kt]: [128, 256] cols = [AT(64) | BT(64) | -BT(64) | AT(64)]
        SIGT = [sigp.tile([128, 256], f32, tag=f"SIGT{i}", name=f"SIGT{i}")
                for i in range(2)]
        for kt in range(2):
            nc.scalar.copy(SIGT[kt][:, 0:64], ATp[kt][:])
            nc.scalar.copy(SIGT[kt][:, 64:128], BTp[kt][:])
            nc.scalar.mul(SIGT[kt][:, 128:192], BTp[kt][:], -1.0)
            nc.scalar.copy(SIGT[kt][:, 192:256], ATp[kt][:])

        # Row-major staging for the outer-product pair regions.  Partition 0
        # of the flat half-tensor FLH holds, per row r, [a | a | b]; partition
        # 1 holds [b | -b | a], giving pairs {a,b}, {a,-b}, {b,a} at free
        # offsets (3*loc + pair) * N.
        SROW = sigp.tile([ROWS, 3 * N], f32, tag="SROW", name="SROW")
        SROW2 = sigp.tile([ROWS, 3 * N], f32, tag="SROW2", name="SROW2")
        for dst_t, i, src_ps, neg in (
                (SROW, 0, A_ps, False), (SROW, 1, A_ps, False),
                (SROW, 2, B_ps, False),
                (SROW2, 0, B_ps, False), (SROW2, 1, B_ps, True),
                (SROW2, 2, A_ps, False)):
            dst = dst_t[:, i * N:(i + 1) * N]
            if neg:
                nc.scalar.mul(dst, src_ps[:], -1.0)
            else:
                nc.scalar.copy(dst, src_ps[:])

        # gate matmul: waits on the LAST Act copy (SROW2 block 2), which
        # absorbs every earlier Act tick (SIGT/SROW copies) into PE's
        # observed clock so later matmuls don't re-wait on Activation.
        gp = psA.tile([1, 256], f32, tag="gateA", name="gateA")
        nc.tensor.matmul(gp[:], SROW2[:, 512:513], SROW2[:, 512:768],
                         start=True, stop=True)
        stA.__exit__(None, None, None)

        # ---------- Stage C: matvecs in transposed layout ----------
        # psum PS[o][pb] [128, 128]: unit u owns cols u*16..u*16+16,
        # cols u*16..+8 = Re part, +8..+16 = Im part.
        #   R  (o=0): lhsT = mrt/mit tiles, Re = mr@a - mi@b, Im = mr@b + mi@a
        #   Rt (o=1): lhsT = mr/mi tiles (contraction over p)
        stC = tc.tile_pool(name="psC", bufs=1, space="PSUM")
        psC = stC.__enter__()
        MASKS = {}
        PS = [[psC.tile([128, 128], f32, tag=f"ps{o}{pb}", name=f"ps{o}{pb}")
               for pb in range(2)] for o in range(2)]

        def ab16(u):
            # 16-col stationary view [A-8 | B-8] of SIGT
            return lambda kt: SIGT[kt][:].rearrange(
                "p (g c) -> p g c", g=4)[:, 0:2, u * B:u * B + B]

        def nba16(u):
            # [-B-8 | A2-8]
            return lambda kt: SIGT[kt][:].rearrange(
                "p (g c) -> p g c", g=4)[:, 2:4, u * B:u * B + B]

        for u in range(UNITS):
            # each mask loads with ONE dma into [128, 512]: cols kt*256+q
            mrt = masktp.tile([128, 512], f32, tag="mrt", name="mrt")
            mit = masktp.tile([128, 512], f32, tag="mit", name="mit")
            mrk = maskp.tile([128, 512], f32, tag=f"mr_{u}", name=f"mr_{u}")
            mik = maskp.tile([128, 512], f32, tag=f"mi_{u}", name=f"mi_{u}")
            for tile_, dram in ((mrt, t["mrt_d"]), (mit, t["mit_d"]),
                                (mrk, t["mr_d"]), (mik, t["mi_d"])):
                nc.sync.dma_start(
                    tile_[:].rearrange("p (kt q) -> p kt q", kt=2),
                    dram[u].rearrange("(kt p) q -> p kt q", kt=2))
                # ifmap-side gate: absorbs this tile's DMA tick into PE's
                # observed clock (MM-side wait; LDW stays clean for later mms)
                gp = psC.tile([1, 512], f32, tag="gateC", name="gateC", bufs=2)
                nc.tensor.matmul(gp[:], SIGT[0][:, 0:1], tile_[:],
                                 start=True, stop=True)
            MASKS[(u, 0, "r")] = mrk
            MASKS[(u, 0, "i")] = mik
            ucol = slice(u * 16, u * 16 + 16)
            for o, (m_re, m_im) in enumerate(((mrt, mit), (mrk, mik))):
                for pb in range(2):
                    out = PS[o][pb][:, ucol]
                    for kt in range(2):
                        pbsl = slice(kt * 256 + pb * 128, kt * 256 + pb * 128 + 128)
                        nc.tensor.matmul(out, m_re[:, pbsl], ab16(u)(kt),
                                         start=(kt == 0), stop=False)
                    for kt in range(2):
                        pbsl = slice(kt * 256 + pb * 128, kt * 256 + pb * 128 + 128)
                        nc.tensor.matmul(out, m_im[:, pbsl], nba16(u)(kt),
                                         start=False, stop=(kt == 1))

        # ---------- Stage D: spectra products + irfft + y ----------
        # ReRST[kt] = AT (.) Re - BT (.) Im ; ImRST[kt] = AT (.) Im + BT (.) Re
        tmp1 = sigp.tile([128, 64], f32, tag="tmp1", name="tmp1")
        tmp2 = sigp.tile([128, 64], f32, tag="tmp2", name="tmp2")
        RST = {}
        for o, nm in ((0, "RS"), (1, "CS")):
            for kt in range(2):
                atv = SIGT[kt][:, 0:64].rearrange("p (u c) -> p u c", c=8)
                btv = SIGT[kt][:, 64:128].rearrange("p (u c) -> p u c", c=8)
                pv = PS[o][kt][:].rearrange("p (u h c) -> p u h c", h=2, c=8)
                re_v, im_v = pv[:, :, 0, :], pv[:, :, 1, :]
                ret = sigp.tile([128, 64], f32, tag=f"re{nm}{kt}", name=f"re{nm}{kt}")
                imt = sigp.tile([128, 64], f32, tag=f"im{nm}{kt}", name=f"im{nm}{kt}")
                t1v = tmp1[:].rearrange("p (u c) -> p u c", c=8)
                t2v = tmp2[:].rearrange("p (u c) -> p u c", c=8)
                nc.vector.tensor_mul(t1v, atv, re_v)
                nc.vector.tensor_mul(t2v, btv, im_v)
                nc.vector.tensor_sub(ret[:], tmp1[:], tmp2[:])
                nc.vector.tensor_mul(t1v, atv, im_v)
                nc.vector.tensor_mul(t2v, btv, re_v)
                nc.vector.tensor_add(imt[:], tmp1[:], tmp2[:])
                RST[(nm, kt)] = (ret, imt)
        stC.__exit__(None, None, None)

        stD = tc.tile_pool(name="psD", bufs=1, space="PSUM")
        psD = stD.__enter__()
        sm_r = psD.tile([ROWS, N], f32, tag="smr", name="smr")
        sm_c = psD.tile([ROWS, N], f32, tag="smc", name="smc")
        for ps, nm in ((sm_r, "RS"), (sm_c, "CS")):
            nc.tensor.matmul(ps[:], RST[(nm, 0)][0][:], WR[0][:], start=True, stop=False)
            nc.tensor.matmul(ps[:], RST[(nm, 1)][0][:], WR[1][:], start=False, stop=False)
            nc.tensor.matmul(ps[:], RST[(nm, 0)][1][:], WI[0][:], start=False, stop=False)
            nc.tensor.matmul(ps[:], RST[(nm, 1)][1][:], WI[1][:], start=False, stop=True)

        G = sigp.tile([ROWS, N], f32, tag="G", name="G")
        G2 = sigp.tile([ROWS, N], f32, tag="G2", name="G2")
        YR = sigp.tile([ROWS, N], f32, tag="YR", name="YR")
        nc.vector.tensor_scalar(G[:], sm_r[:], LAM[:, 0:1], None, op.mult)
        nc.vector.scalar_tensor_tensor(G2[:], sm_c[:], LAM[:, 1:2], G[:],
                                       op.mult, op.add)
        nc.vector.scalar_tensor_tensor(YR[:], XROW[:], LAM[:, 2:3], G2[:],
                                       op.mult, op.add)
        nc.sync.dma_start(t["y_d"][:, :], YR[:])

        # Block-diagonal operand tensors for the outer products (stage E).
        # P_b[p,q] = sum_k lhsT[k,p] rhs[k,q-block b] with k = 16 rows:
        #   rows 0:8  = a_b'[.]   (diag block b' of rhs / column p of lhsT)
        #   rows 8:16 = -b_b'[.] (P)   or   a/b swapped (Q)
        # One K=16, N=512 matmul then yields P (or Q) for TWO batch rows.
        # rhs RD[u%2]: [16, 2048] block-diagonal, refilled per unit by 16
        # small DMAs (zeros persist across refills since slots alternate).
        # P operands live at partitions 0:16 (PE row group 0), Q operands at
        # 64:80 (row group 64): alternating groups lets each LDWEIGHTS
        # overlap the in-flight matmul of the other group.
        RD = [sigp.tile([128, B * N], f32, tag=f"RD{i}", name=f"RD{i}")
              for i in range(2)]
        LTT = [sigp.tile([128, N], f32, tag=f"LTT{i}", name=f"LTT{i}")
               for i in range(2)]
        for i in range(2):
            nc.vector.memset(RD[i][0:16, :], 0.0)
            nc.vector.memset(RD[i][64:80, :], 0.0)
        stD.__exit__(None, None, None)

        # ---------- Stage E: superposition s ----------
        # Flat pair regions (refilled per unit, one DMA per partition):
        # both operands of every outer product start at partition 0.
        #   P = a@a - b@b:  lhsT = {a,b} (region 0),  rhs = {a,-b} (region 1)
        #   Q = a@b + b@a:  lhsT = {a,b},             rhs = {b,a}  (region 2)
        stE = tc.tile_pool(name="psPQ", bufs=2, space="PSUM")
        psPQ = stE.__enter__()
        ngroup = B // BGROUP
        FD = BGROUP * N

        for u in range(UNITS):
            r0 = u * B
            rd, ltt = RD[u % 2], LTT[u % 2]
            # refill block-diagonal rhs + stationaries for this unit
            for bb_ in range(B):
                r = r0 + bb_
                nc.sync.dma_start(rd[bb_:bb_ + 1, bb_ * N:(bb_ + 1) * N],
                                  SROW[r:r + 1, 0:N])
                nc.sync.dma_start(rd[8 + bb_:9 + bb_, bb_ * N:(bb_ + 1) * N],
                                  SROW[r:r + 1, 2 * N:3 * N])
            nc.sync.dma_start(rd[64:80, :], rd[0:16, :])
            nc.sync.dma_start(ltt[0:8, :], SROW[r0:r0 + B, 0:N])
            nc.sync.dma_start(ltt[8:16, :], SROW2[r0:r0 + B, N:2 * N])
            nc.sync.dma_start(ltt[64:72, :], SROW[r0:r0 + B, 2 * N:3 * N])
            nc.sync.dma_start(ltt[72:80, :], SROW[r0:r0 + B, 0:N])
            for pt in range(2):
                psl = slice(pt * 128, pt * 128 + 128)
                for g in range(ngroup):
                    P4 = psPQ.tile([128, FD], f32, tag="P", name="P")
                    Q4 = psPQ.tile([128, FD], f32, tag="Q", name="Q", bufs=1)
                    for h in range(2):
                        rsl = slice(g * FD + h * 512, g * FD + (h + 1) * 512)
                        osl = slice(h * 512, (h + 1) * 512)
                        nc.tensor.matmul(P4[:, osl], ltt[0:16, psl],
                                         rd[0:16, rsl], start=True, stop=True,
                                         tile_position=(0, 0))
                        nc.tensor.matmul(Q4[:, osl], ltt[64:80, psl],
                                         rd[64:80, rsl], start=True, stop=True,
                                         tile_position=(64, 0))
                    mr_pt = MASKS[(u, 0, "r")][:, pt * N:(pt + 1) * N]
                    mi_pt = MASKS[(u, 0, "i")][:, pt * N:(pt + 1) * N]
                    mrb = mr_pt.unsqueeze(1).broadcast_to((128, BGROUP, N))
                    mib = mi_pt.unsqueeze(1).broadcast_to((128, BGROUP, N))
                    t1 = ttp.tile([128, FD], f32, tag="t1", name="t1")
                    t2 = ttp.tile([128, FD], f32, tag="t2", name="t2")
                    nc.vector.tensor_tensor(
                        t1[:].rearrange("p (b q) -> p b q", b=BGROUP),
                        P4[:].rearrange("p (b q) -> p b q", b=BGROUP), mrb, op.mult)
                    nc.vector.tensor_tensor(
                        t2[:].rearrange("p (b q) -> p b q", b=BGROUP),
                        Q4[:].rearrange("p (b q) -> p b q", b=BGROUP), mib, op.mult)
                    s4 = sp.tile([128, FD], f32, tag="s", name="s")
                    nc.gpsimd.tensor_sub(s4[:], t1[:], t2[:])
                    dst = t["s_d"][u, g * BGROUP:(g + 1) * BGROUP,
                                   pt * 128:(pt + 1) * 128, :].rearrange(
                                       "b p q -> p b q")
                    nc.sync.dma_start(dst, s4[:].rearrange("p (b q) -> p b q",
                                                           b=BGROUP))
        stE.__exit__(None, None, None)


_NC_CACHE = None


def _get_nc():
    global _NC_CACHE
    if _NC_CACHE is None:
        _NC_CACHE = build_program()
    return _NC_CACHE


def make_in_maps(x, mask_real, mask_imag, polarization, gauss_mean, gauss_std):
    """Host-side sharding + tiny exact statistics."""
    x = np.asarray(x, np.float32)
    mr_f = np.asarray(mask_real, np.float32)
    mi_f = np.asarray(mask_imag, np.float32)
    pol = np.asarray(polarization, np.float64)
    gm = np.asarray(gauss_mean, np.float64)
    gs = np.asarray(gauss_std, np.float64)

    cos, nsin, wr, wi = _host_constants()

    # exact correlation statistic (DC-bin reduction of the irfft mean)
    d = x.astype(np.float64).sum(-1)                       # [B,S,C]
    corr = np.einsum('bic,bjc->ij', d, d) / (B * C * N)    # [S,S]
    mix = np.exp(-0.5 * ((corr - gm[:, None]) / gs[:, None]) ** 2)
    mixbar = mix.mean(1)                                   # [S]
    lam1 = (mixbar * np.cos(pol)).astype(np.float32)
    lam2 = (mixbar * np.sin(pol)).astype(np.float32)
    lam3 = (1.0 - mixbar).astype(np.float32)

    in_maps = []
    for core in range(NCORES):
        s0 = core * SIG_PER_CORE
        xs = x[:, s0:s0 + SIG_PER_CORE]                    # [B,2,C,N]
        xrow = np.ascontiguousarray(
            xs.transpose(1, 2, 0, 3).reshape(ROWS, N))     # rows r=(sl,c,b)
        xrt = np.ascontiguousarray(xrow.T)                 # [N, ROWS]
        mru = np.ascontiguousarray(
            mr_f[s0:s0 + SIG_PER_CORE].reshape(UNITS, N, N))
        miu = np.ascontiguousarray(
            mi_f[s0:s0 + SIG_PER_CORE].reshape(UNITS, N, N))
        mrtu = np.ascontiguousarray(mru.transpose(0, 2, 1))
        mitu = np.ascontiguousarray(miu.transpose(0, 2, 1))
        sl_idx = np.repeat(np.arange(SIG_PER_CORE), C * B) + s0   # [64] signal ids
        lam = np.stack([lam1[sl_idx], lam2[sl_idx], lam3[sl_idx],
                        np.zeros(ROWS, np.float32)], axis=1).astype(np.float32)
        in_maps.append({
            "xr": xrow, "xrt": xrt, "mr": mru, "mi": miu, "mrt": mrtu,
            "mit": mitu, "cosm": cos, "nsinm": nsin, "wrm": wr, "wim": wi,
            "lam": lam,
        })
    return in_maps


def assemble(results):
    """Gather per-core outputs into full (y, s)."""
    y = np.empty((B, S, C, N), np.float32)
    s = np.empty((B, S, C, N, N), np.float32)
    for core, res in enumerate(results):
        s0 = core * SIG_PER_CORE
        yr = res["y_out"].reshape(SIG_PER_CORE, C, B, N)
        y[:, s0:s0 + SIG_PER_CORE] = yr.transpose(2, 0, 1, 3)
        so = res["s_out"].reshape(SIG_PER_CORE, C, B, N, N)
        s[:, s0:s0 + SIG_PER_CORE] = so.transpose(2, 0, 1, 3, 4)
    return y, s


def kernel(x, mask_real, mask_imag, polarization, gauss_mean, gauss_std):
    from concourse.bass_utils import run_bass_kernel_spmd
    nc = _get_nc()
    in_maps = make_in_maps(x, mask_real, mask_imag, polarization,
                           gauss_mean, gauss_std)
    res = run_bass_kernel_spmd(nc, in_maps, core_ids=list(range(NCORES)))
    return assemble(res.results)
